# revision 91
# baseline (speedup 1.0000x reference)
"""Trainium2 Bass kernel for a 2-layer dense GAT (nn_GAT_87144886436203).

Sharding: row-shard the N=4096 nodes across 8 NeuronCores (512 rows each).
Each core computes attention scores for its row block against all N columns
with the contraction axis j on SBUF partitions, so `att @ Wh` needs no
transposes. Softmax normalization rides a ones-column in the gathered Wh
(the row-sum falls out of the score matmul).

Layer-1 node features are computed SHARDED: each core computes Wh and the
f/g projections for its own 512 nodes and two AllGathers distribute them
(a tiny f/g gather first, so every elementwise pipeline unblocks ~10us in;
the wide [Wh_h|1]x8 gather streams in per-4-chunk pieces that only the
score matmuls wait on).

The per-group score pipeline (group = 4 chunks of 128 j x 512 i, fp16)
computes pm = exp(lrelu(f_i + g_j)) * adj with two engine-complementary
variants, interleaved across unit pairs so ACT / DVE / PE stay ~95% busy:
  V3P: PE builds s = f_i - 1024*(1-adj) per chunk in PSUM (an fp8 identity
       matmul of the penalty matrix + a selector-row matmul broadcasting f),
       then ACT Prelu(psum, bias=g_j) + ACT Exp@2048.  The -1024 penalty
       makes exp underflow to exactly 0 off-edge -- no mask multiply at all.
  VD : DVE-only: A = exp(f)_bcast * exp(g_j)  (tensor_scalar, per-partition
       scalar from exp'd g columns), B likewise with the 0.2-scaled exps,
       pm0 = tt-max(A, B)  (exp(lrelu(s)) == max(exp(s), exp(.2 s))), then
       one DVE mask multiply by the 0/1 adjacency.
Mask+score matmuls are deferred one group behind the elementwise work so
in-order engines never head-of-line block, and the ones-column row-sum
normalizes the masked softmax exactly.

Pool/GPSIMD on real TRN2 cannot execute TensorTensor/TensorScalar or read
PSUM (the BIR verifier rejects them), so it only issues memsets here; all
elementwise work lives on ACT/DVE with PE carrying the penalty/broadcast
matmuls and attention contractions.
"""

import numpy as np
import ml_dtypes

import concourse.bass as bass
import concourse.bacc as bacc
import concourse.tile as tile
import concourse.mybir as mybir
from concourse import masks
from concourse.bass_utils import run_bass_kernel_spmd

F16 = mybir.dt.float16
F32 = mybir.dt.float32
NPF16 = ml_dtypes.float16 if hasattr(ml_dtypes, "float16") else np.float16

NCORES = 8
N = 4096            # nodes
K = 512             # input feature dim (= NFEAT)
H = 8               # heads (layer 1)
D = 64              # per-head hidden (= NHID = NCLASS)
DALL = H * D        # 512
R = N // NCORES     # 512 rows per core
JC = N // 128       # 32 j-chunks
G = 4               # j-chunks per group (free dim 2048 for the big ops)
NG = JC // G        # 8 groups
AUG1 = D + 1        # 65: [Wh_h | ones]
W1S = H * AUG1      # 520
GT1W = W1S           # 520: [Wh|1]x8
AUG2 = D + 2        # 66: [Wh2 | ones | g2]
ALPHA = 0.2

# ---- variant schedule ---------------------------------------------------- #
# V3p: PE builds s = f_i + g-bias + mask-penalty in PSUM (identity/selector
#      matmuls); ACT does Prelu-from-PSUM + Exp.  Maskless: the -1024 penalty
#      drives exp to exactly 0 for non-edges.
# VD : DVE tensor-scalar exp outer-products + tt-max + mask-mult
#      (exp(lrelu(s)) == max(exp(s), exp(.2 s))), no ACT at all.
# V1/V2: ACT / DVE-stock lrelu variants (kept for tuning).
V1, V2, VD, V3P = 0, 1, 2, 3
MASK_POOL = set()
PAIR_SCHED = [
    (V3P, VD), (VD, V3P), (V3P, VD), (VD, V3P),
    (V3P, VD), (VD, V3P), (V3P, VD), (VD, V3P),
]
L2_SCHED = [VD, V3P, VD, VD, VD, V3P, VD, V3P]


def _sched(u, g):
    if u == H:
        return L2_SCHED[g]
    return PAIR_SCHED[g][u % 2]


_CACHE = {}


# --------------------------------------------------------------------------- #
# device program
# --------------------------------------------------------------------------- #

def _build(emulate_collective=False):
    nc = bacc.Bacc(
        "TRN2",
        target_bir_lowering=False,
        debug=False,
        num_devices=1 if emulate_collective else NCORES,
    )

    xrT = nc.dram_tensor("xrT", [K, R], F16, kind="ExternalInput")
    adjB = nc.dram_tensor("adjB", [N, R], F16, kind="ExternalInput")
    adjP = nc.dram_tensor("adjP", [N, R], mybir.dt.float8e5, kind="ExternalInput")
    selAB = nc.dram_tensor("selAB", [16, 16], F32, kind="ExternalInput")
    W_all = nc.dram_tensor("W_all", [K, DALL], F16, kind="ExternalInput")
    wa = nc.dram_tensor("wa", [K, 2 * H], F16, kind="ExternalInput")
    W_out = nc.dram_tensor("W_out", [DALL, D], F16, kind="ExternalInput")
    wa2 = nc.dram_tensor("wa2", [DALL, 2], F16, kind="ExternalInput")
    out = nc.dram_tensor("out", [R, D], F32, kind="ExternalOutput")

    with tile.TileContext(nc) as tc:
        _emit(nc, tc, locals(), emulate_collective)

    nc.compile()
    return nc


def _emit(nc, tc, io, emulate_collective):
    xrT, adjB, W_all, wa, W_out, wa2, out = (
        io["xrT"], io["adjB"], io["W_all"], io["wa"],
        io["W_out"], io["wa2"], io["out"],
    )
    adjP, selAB = io["adjP"], io["selAB"]
    AT = mybir.AluOpType
    AF = mybir.ActivationFunctionType

    from contextlib import ExitStack
    with ExitStack() as ctx:
        res = ctx.enter_context(tc.tile_pool(name="res", bufs=1))
        bank = ctx.enter_context(tc.tile_pool(name="bank", bufs=4, space="PSUM"))
        ppool = ctx.enter_context(tc.tile_pool(name="ppool", bufs=2, space="PSUM"))
        util = ctx.enter_context(tc.tile_pool(name="util", bufs=2, space="PSUM"))
        work = ctx.enter_context(tc.tile_pool(name="work", bufs=5))
        workm = ctx.enter_context(tc.tile_pool(name="workm", bufs=4))
        work2 = ctx.enter_context(tc.tile_pool(name="work2", bufs=5))
        small = ctx.enter_context(tc.tile_pool(name="small", bufs=4))
        rpool = ctx.enter_context(tc.tile_pool(name="rpool", bufs=2))
        dram = ctx.enter_context(tc.tile_pool(name="dram", bufs=1, space="DRAM"))

        # ---- resident SBUF tensors ---- #
        xrT_sb = res.tile([128, 4 * R], F16, tag="xrT")
        adjB_sb = res.tile([128, JC * R], F16, tag="adjB")
        adjP_sb = res.tile([128, JC * R], mybir.dt.float8e5, tag="adjP")
        identE5_sb = res.tile([128, 128], mybir.dt.float8e5, tag="identE5")
        selL_sb = res.tile([16, H * 128], F16, tag="selL")
        sel2L_sb = res.tile([2, 128], F16, tag="sel2L")
        onesL_sb = res.tile([16, 128], F16, tag="onesL")
        selAB_sb = res.tile([16, 16], F32, tag="selAB")
        W_sb = res.tile([128, 4 * DALL], F16, tag="W")
        wa_sb = res.tile([128, 4 * 2 * H], F16, tag="wa")
        W_out_sb = res.tile([128, 4 * D], F16, tag="W_out")
        wa2_sb = res.tile([128, 4 * 2], F16, tag="wa2")
        gt1_sb = res.tile([128, 4 * GT1W], F16, tag="gt1")      # pre-gather
        gtfg_sb = res.tile([128, 4 * 2 * H], F16, tag="gtfg")   # my fg rows
        fg16_sb = res.tile([128, JC * 2 * H], F16, tag="fg16")  # gathered fg
        whb_sb = res.tile([128, JC * GT1W], F16, tag="whb")     # gathered
        fg32_sb = res.tile([128, JC * 2 * H], F32, tag="fg32")  # g scalars
        E12g_sb = res.tile([128, JC * 16], F32, tag="E12g")     # packed g-exps
        EfA2_sb = res.tile([128, H * R], F16, tag="EfA2")       # exp(.2 frepall)
        fgr16_sb = res.tile([16, R], F16, tag="fgr16")
        frepall_sb = res.tile([128, H * R], F16, tag="frepall")
        frep2_sb = res.tile([128, R], F16, tag="frep2")
        hcatT_sb = res.tile([128, 4 * R], F16, tag="hcatT")
        whb2_sb = res.tile([128, JC * AUG2], F16, tag="whb2")
        g2_sb = res.tile([128, JC], F32, tag="g2")
        Efp2a_sb = res.tile([128, R], F16, tag="Efp2a")         # exp(frep2)
        Efp2b_sb = res.tile([128, R], F16, tag="Efp2b")
        eg2a_sb = res.tile([128, JC], F32, tag="eg2a")          # exp(g2 cols)
        eg2b_sb = res.tile([128, JC], F32, tag="eg2b")
        fg2_sb = res.tile([2, R], F32, tag="fg2")
        ones_sb = res.tile([1, 128], F32, tag="ones")
        ones16_sb = res.tile([1, 128], F16, tag="ones16")
        ident_sb = res.tile([128, 128], F16, tag="ident")
        ident32_sb = res.tile([64, 64], F32, tag="ident32")
        out_sb = res.tile([128, 4 * D], F32, tag="out_sb")

        def chunked(dram_t, width):
            return dram_t.ap().rearrange("(c p) w -> p c w", p=128)

        def chunked_sb(sb_ap, width):
            return sb_ap.rearrange("p (c w) -> p c w", w=width)

        def load(sb_tile, dram_t, width, split=1, split_free=1, eng=None):
            eng = eng or nc.sync
            dst = chunked_sb(sb_tile[:], width)
            src = chunked(dram_t, width)
            nch = dst.shape[1]
            step = max(1, nch // split)
            fstep = max(1, width // split_free)
            for lo in range(0, nch, step):
                hi = min(nch, lo + step)
                for flo in range(0, width, fstep):
                    fhi = min(width, flo + fstep)
                    eng.dma_start(dst[:, lo:hi, flo:fhi],
                                  src[:, lo:hi, flo:fhi])

        # ---- phase 0: loads + constants ---- #
        load(xrT_sb, xrT, R)
        load(wa_sb, wa, 2 * H)
        load(W_sb, W_all, DALL)
        load(W_out_sb, W_out, D)
        load(wa2_sb, wa2, 2)
        nc.vector.memset(ones_sb[:], 1.0)
        nc.vector.memset(ones16_sb[:], 1.0)
        nc.vector.memset(onesL_sb[:], 1.0)
        # PE warmup: ~2us of dummy matmuls while inputs stream in, so the
        # prep matmuls start at full p-state instead of 0.65 GHz
        pwarm = bank.tile([128, 128], F32, tag="bank")
        for w in range(24):
            nc.tensor.matmul(pwarm[:], onesL_sb[:], onesL_sb[:],
                             start=(w == 0), stop=(w == 23))
        nc.sync.dma_start(selAB_sb[:], selAB.ap())
        masks.make_identity(nc, identE5_sb[:])
        for h in range(H):
            nc.vector.tensor_scalar(
                selL_sb[:, h * 128:(h + 1) * 128], onesL_sb[:],
                selAB_sb[:, h:h + 1], None, AT.mult)
        nc.vector.tensor_scalar(
            sel2L_sb[:], onesL_sb[0:2, :], selAB_sb[0:2, 0:1], None, AT.mult)
        masks.make_identity(nc, ident_sb[:])
        masks.make_identity(nc, ident32_sb[:])

        # ---- phase 1: sharded Wh/fg prep + AllGather ---- #
        # f/g projections of my rows: [16, R] (row 2h = f_h, 2h+1 = g_h)
        pfr = util.tile([16, R], F32, tag="u")
        for kc in range(4):
            nc.tensor.matmul(
                pfr[:], wa_sb[:, kc * 2 * H:(kc + 1) * 2 * H],
                xrT_sb[:, kc * R:(kc + 1) * R],
                start=(kc == 0), stop=(kc == 3),
            )
        nc.vector.tensor_copy(fgr16_sb[:], pfr[:])
        fgb_d = dram.tile([16, R], F16, tag="fgb")
        nc.sync.dma_start(fgb_d[:], fgr16_sb[:])
        # broadcast all 8 heads' f-rows to all partitions in one DMA
        fgb_f = fgb_d[:].rearrange("(u t) w -> t u w", t=2)[0:1, :, :]
        nc.sync.dma_start(
            frepall_sb[:].rearrange("p (u w) -> p u w", w=R),
            fgb_f.broadcast_to([128, H, R]))
        nc.scalar.activation(EfA2_sb[:], frepall_sb[:], AF.Exp, scale=ALPHA)
        nc.scalar.activation(frepall_sb[:], frepall_sb[:], AF.Exp)
        EfA1_sb = frepall_sb    # exp(frepall), computed in place


        # fg projections first (fg gather unblocks all elementwise work)
        cc_space = {} if emulate_collective else {"addr_space": "Shared"}
        for ib in range(4):
            pf = bank.tile([128, 2 * H], F32, tag="bank")
            for kc in range(4):
                lhsT = xrT_sb[:, kc * R + ib * 128: kc * R + (ib + 1) * 128]
                nc.tensor.matmul(
                    pf[:], lhsT, wa_sb[:, kc * 2 * H:(kc + 1) * 2 * H],
                    start=(kc == 0), stop=(kc == 3))
            nc.vector.tensor_copy(
                gtfg_sb[:, ib * 2 * H:(ib + 1) * 2 * H], pf[:])
        ccf_in = dram.tile([R, 2 * H], F16, tag="ccf_in")
        ccf_out = dram.tile([N, 2 * H], F16, tag="ccf_out", **cc_space)
        ccf_out_ch = ccf_out[:].rearrange("(c p) w -> p c w", p=128)
        fg16_ch = chunked_sb(fg16_sb[:], 2 * H)
        if emulate_collective:
            for b in range(NG):
                cl, ch_ = b * G, (b + 1) * G
                nc.sync.dma_start(
                    ccf_out_ch[:, cl:ch_, :], chunked_sb(gtfg_sb[:], 2 * H))
                nc.sync.dma_start(fg16_ch[:, cl:ch_, :],
                                    ccf_out_ch[:, cl:ch_, :])
        else:
            nc.sync.dma_start(
                ccf_in[:].rearrange("(c p) w -> p c w", p=128),
                chunked_sb(gtfg_sb[:], 2 * H))
            nc.gpsimd.collective_compute(
                "AllGather", mybir.AluOpType.bypass,
                replica_groups=[list(range(NCORES))],
                ins=[ccf_in.opt()], outs=[ccf_out.opt()],
            )
            nc.sync.dma_start(fg16_ch[:], ccf_out_ch[:])

        # gt1 = [Wh_h|1]x8 for my 512 rows
        nc.gpsimd.memset(gt1_sb[:], 1.0)
        for ib in range(4):
            pw = bank.tile([128, DALL], F32, tag="bank")
            for kc in range(4):
                lhsT = xrT_sb[:, kc * R + ib * 128: kc * R + (ib + 1) * 128]
                nc.tensor.matmul(
                    pw[:], lhsT, W_sb[:, kc * DALL:(kc + 1) * DALL],
                    start=(kc == 0), stop=(kc == 3))
            dst = gt1_sb[:, ib * GT1W: ib * GT1W + W1S].rearrange(
                "p (h x) -> p h x", x=AUG1)[:, :, 0:D]
            nc.vector.tensor_copy(dst, pw.rearrange("p (h x) -> p h x", x=D))

        cc1_in = dram.tile([R, GT1W], F16, tag="cc1_in")
        cc1_out = dram.tile([N, GT1W], F16, tag="cc1_out", **cc_space)
        cc1_in_ch = cc1_in[:].rearrange("(c p) w -> p c w", p=128)
        if emulate_collective:
            pass
        else:
            for ib in range(4):
                nc.sync.dma_start(
                    cc1_in_ch[:, ib:ib + 1, :],
                    chunked_sb(gt1_sb[:], GT1W)[:, ib:ib + 1, :])
            nc.gpsimd.collective_compute(
                "AllGather", mybir.AluOpType.bypass,
                replica_groups=[list(range(NCORES))],
                ins=[cc1_in.opt()], outs=[cc1_out.opt()],
            )
        whb_ch = chunked_sb(whb_sb[:], GT1W)
        cc1_out_ch = cc1_out[:].rearrange("(c p) w -> p c w", p=128)
        fg_g = chunked_sb(fg32_sb[:], 2 * H).rearrange(
            "p c (h t) -> p c h t", t=2)[:, :, :, 1]
        E12v = chunked_sb(E12g_sb[:], 16).rearrange(
            "p c (h t) -> p c h t", t=2)
        adjB_ch = chunked(adjB, R)
        adjB_dst = chunked_sb(adjB_sb[:], R)
        adjP_ch = chunked(adjP, R)
        adjP_dst = chunked_sb(adjP_sb[:], R)

        def cp_eng(i, dst, src):
            if i % 2 == 0:
                nc.vector.tensor_copy(dst, src)
            else:
                nc.scalar.activation(dst, src, AF.Copy)

        # fg-derived prep per piece (fg gather is tiny and lands early)
        for b in range(NG):
            cl, ch_ = b * G, (b + 1) * G
            nc.sync.dma_start(adjP_dst[:, cl:ch_, :], adjP_ch[:, cl:ch_, :])
            nc.sync.dma_start(adjB_dst[:, cl:ch_, :], adjB_ch[:, cl:ch_, :])
            nc.vector.tensor_copy(
                chunked_sb(fg32_sb[:], 2 * H)[:, cl:ch_, :],
                fg16_ch[:, cl:ch_, :])
            nc.scalar.activation(E12v[:, cl:ch_, :, 0], fg_g[:, cl:ch_],
                                 AF.Exp)
            nc.scalar.activation(E12v[:, cl:ch_, :, 1], fg_g[:, cl:ch_],
                                 AF.Exp, scale=ALPHA)

        # wh gather pieces, interleaved into pair 0's group loop
        def piece_prep(b):
            cl, ch_ = b * G, (b + 1) * G
            if emulate_collective:
                nc.sync.dma_start(
                    cc1_out_ch[:, cl:ch_, :], chunked_sb(gt1_sb[:], GT1W))
            nc.sync.dma_start(whb_ch[:, cl:ch_, :],
                                cc1_out_ch[:, cl:ch_, :])

        # ---- attention machinery ---- #

        def unit_ew(u, g, lhsT_of, g_of, frep_t, rows):
            """Elementwise part; returns pmm for deferred score mms."""
            v = _sched(u, g)
            pm = work.tile([128, G * R], F16, tag="pm")
            if v == V3P:
                efA, efB, egc1, egc2, fsel, frows = rows[0], rows[1], rows[2], rows[3], rows[4], rows[5]
                uu = work2.tile([128, G * R], F16, tag="uu")
                for c in range(G):
                    jc = g * G + c
                    psS = bank.tile([128, R], F32, tag="bank")
                    nc.tensor.matmul(
                        psS[:], identE5_sb[:],
                        adjP_sb[:, jc * R:(jc + 1) * R],
                        start=True, stop=False)
                    nc.tensor.matmul(
                        psS[:], fsel, frows, start=False, stop=True)
                    nc.scalar.activation(
                        uu[:, c * R:(c + 1) * R], psS[:],
                        AF.Prelu, bias=g_of(jc), alpha=ALPHA)
                nc.scalar.activation(pm[:], uu[:], AF.Exp)
            elif v == V1:
                uu = work2.tile([128, G * R], F16, tag="uu")
                for c in range(G):
                    jc = g * G + c
                    nc.scalar.activation(
                        uu[:, c * R:(c + 1) * R], frep_t,
                        AF.Prelu, bias=g_of(jc), alpha=ALPHA)
                nc.scalar.activation(pm[:], uu[:], AF.Exp)
            elif v == V2:
                s = work2.tile([128, G * R], F16, tag="uu")
                for c in range(G):
                    jc = g * G + c
                    nc.vector.tensor_scalar(
                        s[:, c * R:(c + 1) * R], frep_t,
                        g_of(jc), None, AT.add)
                t = work2.tile([128, G * R], F16, tag="uu")
                nc.vector.tensor_scalar(t[:], s[:], ALPHA, None, AT.mult)
                nc.vector.tensor_tensor(t[:], s[:], t[:], AT.max)
                nc.scalar.activation(pm[:], t[:], AF.Exp)
            else:  # VD: ts outer products in SBUF + DVE max
                efA, efB, egc1, egc2 = rows[0], rows[1], rows[2], rows[3]
                abA = work2.tile([128, G * R], F16, tag="uu")
                abB = work2.tile([128, G * R], F16, tag="uu")
                for c in range(G):
                    jc = g * G + c
                    nc.vector.tensor_scalar(
                        abA[:, c * R:(c + 1) * R], efA, egc1(jc), None,
                        AT.mult)
                    nc.vector.tensor_scalar(
                        abB[:, c * R:(c + 1) * R], efB, egc2(jc), None,
                        AT.mult)
                nc.vector.tensor_tensor(pm[:], abA[:], abB[:], AT.max)
            return pm

        def unit_fin(u, g, pout, lhsT_of, pm):
            """Deferred mask + score matmuls for group g."""
            v = _sched(u, g)
            if v == V3P:
                pmm = pm        # penalty already zeroed the non-edges
            else:
                pmm = workm.tile([128, G * R], F16, tag="pmm")
                nc.vector.tensor_tensor(
                    pmm[:], pm[:], adjB_sb[:, g * G * R:(g + 1) * G * R],
                    AT.mult)
            for c in range(G):
                jc = g * G + c
                nc.tensor.matmul(
                    pout[:], lhsT_of(jc), pmm[:, c * R:(c + 1) * R],
                    start=(jc == 0), stop=(jc == JC - 1))

        def epilogue(pout, dst_ap, dst_f32):
            """dst = elu(att_out / rowsum) written to dst_ap ([64, R])."""
            dt = F32 if dst_f32 else F16
            recip = rpool.tile([1, R], F32, tag="recip")
            nc.vector.reciprocal(recip[:], pout[D:D + 1, :])
            pr = util.tile([D, R], F32, tag="u")
            nc.tensor.matmul(pr[:], ones_sb[0:1, 0:D], recip[:],
                             start=True, stop=True)
            rsb = small.tile([D, R], F32, tag="ep")
            nc.any.tensor_copy(rsb[:], pr[:])
            hl = small.tile([D, R], dt, tag="ep")
            nc.vector.tensor_tensor(hl[:], pout[0:D, :], rsb[:], AT.mult)
            # elu(x) = max(x,0) + min(exp(x),1) - 1   (exp monotone)
            q = small.tile([D, R], dt, tag="ep")
            nc.scalar.activation(q[:], hl[:], AF.Exp)
            t1 = small.tile([D, R], dt, tag="ep")
            nc.vector.tensor_scalar(t1[:], q[:], 1.0, -1.0, AT.min, AT.add)
            t2 = small.tile([D, R], dt, tag="ep")
            nc.vector.tensor_scalar(t2[:], hl[:], 0.0, None, AT.max)
            nc.vector.tensor_tensor(dst_ap, t1[:], t2[:], AT.add)

        # ---- phase 2: layer-1 attention units, pair-interleaved ---- #
        def l1_args(h):
            lhsT_of = lambda jc: whb_sb[
                :, jc * GT1W + h * AUG1: jc * GT1W + (h + 1) * AUG1]
            g_of = lambda jc: fg32_sb[
                :, jc * 2 * H + 2 * h + 1: jc * 2 * H + 2 * h + 2]
            rows = (EfA1_sb[:, h * R:(h + 1) * R],
                    EfA2_sb[:, h * R:(h + 1) * R],
                    lambda jc, h=h: E12g_sb[
                        :, jc * 16 + 2 * h: jc * 16 + 2 * h + 1],
                    lambda jc, h=h: E12g_sb[
                        :, jc * 16 + 2 * h + 1: jc * 16 + 2 * h + 2],
                    selL_sb[:, h * 128:(h + 1) * 128],
                    fgr16_sb[:])
            frep_t = frepall_sb[:, h * R:(h + 1) * R]
            return lhsT_of, g_of, rows, frep_t

        for p in range(H // 2):
            ua, ub = 2 * p, 2 * p + 1
            argsa, argsb = l1_args(ua), l1_args(ub)
            pouta = ppool.tile([AUG1, R], F32, tag="pout")
            poutb = ppool.tile([AUG1, R], F32, tag="pout")
            pend = None
            for g in range(NG):
                if p == 0:
                    piece_prep(g)
                if pend is not None:
                    unit_fin(ua, g - 1, pouta, argsa[0], pend[0])
                    unit_fin(ub, g - 1, poutb, argsb[0], pend[1])
                pma = unit_ew(ua, g, argsa[0], argsa[1], argsa[3], argsa[2])
                pmb = unit_ew(ub, g, argsb[0], argsb[1], argsb[3], argsb[2])
                pend = (pma, pmb)
            unit_fin(ua, NG - 1, pouta, argsa[0], pend[0])
            unit_fin(ub, NG - 1, poutb, argsb[0], pend[1])
            for h, pout in ((ua, pouta), (ub, poutb)):
                kc, po = h // 2, (h % 2) * D
                epilogue(pout, hcatT_sb[po:po + D, kc * R:(kc + 1) * R],
                         dst_f32=False)

        # ---- phase 3: layer-2 prep + allgather ---- #
        gt2_sb = res.tile([128, 4 * AUG2], F16, tag="gt2")
        nc.vector.memset(gt2_sb[:], 1.0)   # ones column comes for free
        for ib in range(4):
            pw2 = bank.tile([128, D], F32, tag="bank")
            pg2 = bank.tile([128, 2], F32, tag="bank")
            for kc in range(4):
                lhsT = hcatT_sb[:, kc * R + ib * 128: kc * R + (ib + 1) * 128]
                nc.tensor.matmul(pw2[:], lhsT, W_out_sb[:, kc * D:(kc + 1) * D],
                                 start=(kc == 0), stop=(kc == 3))
                nc.tensor.matmul(pg2[:], lhsT, wa2_sb[:, kc * 2:(kc + 1) * 2],
                                 start=(kc == 0), stop=(kc == 3))
            nc.vector.tensor_copy(gt2_sb[:, ib * AUG2: ib * AUG2 + D], pw2[:])
            nc.vector.tensor_copy(
                gt2_sb[:, ib * AUG2 + D + 1: ib * AUG2 + D + 2], pg2[:, 1:2])

        pfg2 = util.tile([2, R], F32, tag="u")
        for kc in range(4):
            nc.tensor.matmul(pfg2[:], wa2_sb[:, kc * 2:(kc + 1) * 2],
                             hcatT_sb[:, kc * R:(kc + 1) * R],
                             start=(kc == 0), stop=(kc == 3))
        nc.vector.tensor_copy(fg2_sb[:], pfg2[:])
        fg2_16 = small.tile([2, R], F16, tag="ep")
        nc.vector.tensor_copy(fg2_16[:], fg2_sb[:])
        pfrep2 = util.tile([128, R], F32, tag="u")
        nc.tensor.matmul(pfrep2[:], ones16_sb[0:1, :], fg2_16[0:1, :],
                         start=True, stop=True)
        nc.vector.tensor_copy(frep2_sb[:], pfrep2[:])
        nc.scalar.activation(Efp2a_sb[:], frep2_sb[:], AF.Exp)
        nc.scalar.activation(Efp2b_sb[:], frep2_sb[:], AF.Exp, scale=ALPHA)

        cc2_in = dram.tile([R, AUG2], F16, tag="cc2_in")
        cc2_out = dram.tile([N, AUG2], F16, tag="cc2_out", **cc_space)
        if emulate_collective:
            pass
        else:
            nc.sync.dma_start(
                cc2_in[:].rearrange("(c p) w -> p c w", p=128),
                chunked_sb(gt2_sb[:], AUG2))
            nc.gpsimd.collective_compute(
                "AllGather", mybir.AluOpType.bypass,
                replica_groups=[list(range(NCORES))],
                ins=[cc2_in.opt()], outs=[cc2_out.opt()],
            )
        whb2_ch = chunked_sb(whb2_sb[:], AUG2)
        cc2_out_ch = cc2_out[:].rearrange("(c p) w -> p c w", p=128)
        g2_ch = g2_sb[:].rearrange("p (c w) -> p c w", w=1)
        def piece_prep2(b):
            lo, hi = b * G, (b + 1) * G
            if emulate_collective:
                nc.sync.dma_start(
                    cc2_out_ch[:, lo:hi, :], chunked_sb(gt2_sb[:], AUG2))
            nc.sync.dma_start(whb2_ch[:, lo:hi, :], cc2_out_ch[:, lo:hi, :])
            nc.vector.tensor_copy(
                g2_ch[:, lo:hi, :], whb2_ch[:, lo:hi, D + 1: D + 2])
            nc.scalar.activation(eg2a_sb[:, lo:hi], g2_sb[:, lo:hi], AF.Exp)
            nc.scalar.activation(eg2b_sb[:, lo:hi], g2_sb[:, lo:hi], AF.Exp,
                                 scale=ALPHA)

        # ---- phase 4: layer 2 ---- #
        pout2 = ppool.tile([AUG1, R], F32, tag="pout")
        l2_lhsT = lambda jc: whb2_sb[:, jc * AUG2: jc * AUG2 + AUG1]
        l2_g = lambda jc: g2_sb[:, jc: jc + 1]
        fg2_16l = res.tile([2, R], F16, tag="fg2_16l")
        nc.vector.tensor_copy(fg2_16l[:], fg2_sb[:])
        rows2 = (Efp2a_sb[0:128, :], Efp2b_sb[0:128, :],
                 lambda jc: eg2a_sb[:, jc:jc + 1],
                 lambda jc: eg2b_sb[:, jc:jc + 1],
                 sel2L_sb[:], fg2_16l[:])
        pend2 = None
        for g in range(NG):
            piece_prep2(g)
            if pend2 is not None:
                unit_fin(H, g - 1, pout2, l2_lhsT, pend2)
            pm2 = unit_ew(H, g, l2_lhsT, l2_g, frep2_sb[0:128, :], rows2)
            pend2 = pm2
        unit_fin(H, NG - 1, pout2, l2_lhsT, pend2)
        res2 = res.tile([D, R], F32, tag="res2")
        epilogue(pout2, res2[:], dst_f32=True)
        for ib in range(4):
            pt = util.tile([128, D], F32, tag="u")
            nc.tensor.transpose(
                pt[:], res2[:, ib * 128:(ib + 1) * 128], ident32_sb[:])
            nc.vector.tensor_copy(out_sb[:, ib * D:(ib + 1) * D], pt[:])
        nc.sync.dma_start(
            out.ap().rearrange("(c p) w -> p c w", p=128),
            chunked_sb(out_sb[:], D))


# --------------------------------------------------------------------------- #
# host side
# --------------------------------------------------------------------------- #

def _pack_inputs(x, adj, W_heads, a_src, a_dst, W_out, a_src_out, a_dst_out):
    """Shard + repack the full inputs into the 8 per-core input maps."""
    x = np.asarray(x, np.float32)
    adj = np.asarray(adj)
    W_heads = np.asarray(W_heads, np.float32)
    a_src = np.asarray(a_src, np.float32)
    a_dst = np.asarray(a_dst, np.float32)
    W_out_np = np.asarray(W_out, np.float32)
    a_src_out = np.asarray(a_src_out, np.float32)
    a_dst_out = np.asarray(a_dst_out, np.float32)

    f16 = NPF16
    W_all = np.ascontiguousarray(
        W_heads.transpose(1, 0, 2).reshape(K, DALL)).astype(f16)     # [K, H*D]
    wa_cols = []
    for h in range(H):
        wa_cols.append(W_heads[h] @ a_src[h])
        wa_cols.append(W_heads[h] @ a_dst[h])
    wa = np.stack(wa_cols, axis=1).astype(f16)                       # [K, 16]
    W_out_p = W_out_np.astype(f16)                                   # [DALL, D]
    wa2 = np.stack([W_out_np @ a_src_out, W_out_np @ a_dst_out],
                   axis=1).astype(f16)                               # [DALL, 2]

    selAB = np.zeros((16, 16), np.float32)
    for u in range(H):
        selAB[2 * u, u] = 1.0
    in_maps = []
    for c in range(NCORES):
        rows = slice(c * R, (c + 1) * R)
        adj_rows = (adj[rows, :] > 0).astype(np.float32)             # [R, N]
        adjB = np.ascontiguousarray(adj_rows.T).astype(f16)          # [N, R] 0/1
        adjP = np.ascontiguousarray(
            (adj_rows.T - 1.0) * 1024.0).astype(ml_dtypes.float8_e5m2)
        in_maps.append({
            "xrT": np.ascontiguousarray(x[rows].T).astype(f16),
            "adjB": adjB,
            "W_all": W_all,
            "wa": wa,
            "W_out": W_out_p,
            "wa2": wa2,
            "adjP": adjP,
            "selAB": selAB,
        })
    return in_maps


def kernel(**inputs) -> np.ndarray:
    if "nc" not in _CACHE:
        _CACHE["nc"] = _build(emulate_collective=False)
    nc = _CACHE["nc"]
    in_maps = _pack_inputs(**inputs)
    res = run_bass_kernel_spmd(nc, in_maps, core_ids=list(range(NCORES)))
    return np.concatenate([res.results[c]["out"] for c in range(NCORES)], axis=0)
